# revision 11
# baseline (speedup 1.0000x reference)
"""BertAttention (B=2,S=2048,D=1024,H=16) on 8 trn2 NeuronCores.

Sharding: data-parallel over B (2 groups of 4 cores), and each group's 4
cores split the 2048 query rows (512 each). Every core computes K^T and V
for its batch in full (redundantly within the group), its own 512-row Q
slice, attention for all 16 heads over its rows, the output projection,
residual add and LayerNorm for its rows. No collectives needed; each core
emits a disjoint [512, 1024] slice of the output.

Layout notes (per core):
  - activations are kept feature-major ("T layout", [D_part, S_free]) so
    every linear layer contracts over SBUF partitions.
  - scores are computed transposed ([ks, qs]); softmax uses the max-free
    trick (scores/8 in [-3.6, 3.6] for this problem scale) so exp is a
    single fused ACT op (scale=1/8) and the attention-mask enters as a
    per-ks multiplicative factor exp(mask) folded into V's rows.
  - V is stored row-scaled by exp(mask) with an extra exp(mask) column per
    head so the PV matmul also produces the softmax denominator (row 64).
  - V is bounced through DRAM to keep SBUF under the 224KB/partition cap.
"""

import numpy as np

B, S, D, H = 2, 2048, 1024, 16
HD = D // H  # 64
P = 128
NCORES = 8
SQ = S // 4  # 512 query rows per core
DT = D // P  # 8 feature tiles
KS = S // P  # 16 key tiles
EPS = 1e-12

_CACHE = {}


def _ensure_paths():
    try:
        import concourse  # noqa: F401
    except ImportError:
        import sys

        for p in ("/opt/trn_rl_repo", "/root/.axon_site/_ro/trn_rl_repo"):
            if p not in sys.path:
                sys.path.append(p)
        import concourse  # noqa: F401


def build_nc():
    """Build the (single, SPMD) bass program."""
    _ensure_paths()
    import concourse.tile as tile
    from concourse import bacc, mybir

    f32 = mybir.dt.float32
    f32r = mybir.dt.float32r

    nc = bacc.Bacc()

    # ---- I/O ----
    xT = nc.declare_dram_parameter("xT", [D, S], f32r, isOutput=False)
    xqT = nc.declare_dram_parameter("xqT", [D, SQ], f32r, isOutput=False)
    xq = nc.declare_dram_parameter("xq", [SQ, D], f32, isOutput=False)
    Wq = nc.declare_dram_parameter("Wq", [D, D], f32r, isOutput=False)
    Wk = nc.declare_dram_parameter("Wk", [D, D], f32r, isOutput=False)
    Wv = nc.declare_dram_parameter("Wv", [D, D], f32r, isOutput=False)
    Wo = nc.declare_dram_parameter("Wo", [D, D], f32r, isOutput=False)
    bq_t = nc.declare_dram_parameter("bq_t", [P, DT], f32, isOutput=False)
    bk_t = nc.declare_dram_parameter("bk_t", [P, DT], f32, isOutput=False)
    bv_bc = nc.declare_dram_parameter("bv_bc", [P, D], f32, isOutput=False)
    bo_bc = nc.declare_dram_parameter("bo_bc", [P, D], f32, isOutput=False)
    gamma_bc = nc.declare_dram_parameter("gamma_bc", [P, D], f32, isOutput=False)
    beta_bc = nc.declare_dram_parameter("beta_bc", [P, D], f32, isOutput=False)
    # exp(attention_mask) laid out [p, kstile]
    emask_t = nc.declare_dram_parameter("emask_t", [P, KS], f32, isOutput=False)
    out = nc.declare_dram_parameter("out", [SQ, D], f32, isOutput=True)

    # internal DRAM bounce for V, layout [stile, p, head, col(65)]
    v_dram = nc.dram_tensor("v_bounce", [KS, P, H, HD + 1], f32r)
    # per-head softmax denominators (bounced through DRAM for partition bcast)
    sums_dram = nc.dram_tensor("sums_bounce", [H, SQ], f32)

    def mm(ps, lhsT, rhs, start, stop):
        nc.tensor.matmul(ps, lhsT, rhs, start=start, stop=stop)


    # rearranged DRAM views
    xT_r = xT.rearrange("(t p) s -> p t s", p=P)  # [128, 8, 2048]
    xqT_r = xqT.rearrange("(t p) s -> p t s", p=P)  # [128, 8, 512]
    xq_r = xq.rearrange("(t p) d -> p t d", p=P)  # [128, 4, 1024]
    W_r = {
        "q": Wq.rearrange("(t p) d -> p t d", p=P),
        "k": Wk.rearrange("(t p) d -> p t d", p=P),
        "v": Wv.rearrange("(t p) d -> p t d", p=P),
        "o": Wo.rearrange("(t p) d -> p t d", p=P),
    }
    out_r = out.rearrange("(t p) d -> t p d", p=P)  # [4, 128, 1024]

    with tile.TileContext(nc) as tc:
        with (
            tc.tile_pool(name="consts", bufs=1) as consts,
            tc.tile_pool(name="pers", bufs=1) as pers,
        ):
            # constants
            bq_sb = consts.tile([P, DT], f32)
            nc.sync.dma_start(bq_sb[:], bq_t[:])
            bk_sb = consts.tile([P, DT], f32)
            nc.sync.dma_start(bk_sb[:], bk_t[:])
            em_sb = consts.tile([P, KS], f32)
            nc.sync.dma_start(em_sb[:], emask_t[:])
            bv_sb = consts.tile([P, D], f32)
            nc.sync.dma_start(bv_sb[:], bv_bc[:])

            # persistent activations
            qt_sb = pers.tile([P, DT, SQ], f32r)  # Q^T   [d, qs]
            kt_sb = pers.tile([P, DT, S], f32r)  # K^T   [d, ks]
            ctxn = pers.tile([P, DT, SQ], f32r)  # ctx^T normalized [d, qs]

            # ---------- Phase Q: QT = Wq^T @ xq ----------
            wctx = tc.tile_pool(name="wpool", bufs=2)
            wpool = wctx.__enter__()
            wq = wpool.tile([P, DT, D], f32r, tag="W")
            nc.sync.dma_start(wq[:], W_r["q"][:])
            with tc.tile_pool(name="ps_qkv", bufs=4, space="PSUM") as ps_pool:
                with tc.tile_pool(name="xqt", bufs=1) as xqt_pool:
                    xqt = xqt_pool.tile([P, DT, SQ], f32r)
                    nc.sync.dma_start(xqt[:], xqT_r[:])
                    for dt in range(DT):
                        ps = ps_pool.tile([P, SQ], f32)
                        for kt in range(DT):
                            mm(
                                ps[:],
                                wq[:, kt, dt * P : (dt + 1) * P],
                                xqt[:, kt, :],
                                start=(kt == 0),
                                stop=(kt == DT - 1),
                            )
                        nc.vector.tensor_scalar_add(
                            qt_sb[:, dt, :], in0=ps[:], scalar1=bq_sb[:, dt : dt + 1]
                        )

                # ---------- Phase KV ----------
                wk = wpool.tile([P, DT, D], f32r, tag="W")
                nc.sync.dma_start(wk[:], W_r["k"][:])
                wv = wpool.tile([P, DT, D], f32r, tag="W")
                nc.sync.dma_start(wv[:], W_r["v"][:])
                with (
                    tc.tile_pool(name="xt", bufs=2) as xt_pool,
                    tc.tile_pool(name="vstage", bufs=2) as vs_pool,
                ):
                    for qtr in range(4):
                        sl = slice(qtr * SQ, (qtr + 1) * SQ)
                        xt_q = xt_pool.tile([P, DT, SQ], f32r)
                        nc.sync.dma_start(xt_q[:], xT_r[:, :, sl])
                        # K^T columns for this quarter
                        for dt in range(DT):
                            ps = ps_pool.tile([P, SQ], f32)
                            for kt in range(DT):
                                mm(
                                    ps[:],
                                    wk[:, kt, dt * P : (dt + 1) * P],
                                    xt_q[:, kt, :],
                                    start=(kt == 0),
                                    stop=(kt == DT - 1),
                                )
                            nc.vector.tensor_scalar_add(
                                kt_sb[:, dt, sl], in0=ps[:], scalar1=bk_sb[:, dt : dt + 1]
                            )
                        # V rows (natural layout) for this quarter
                        for st4 in range(4):
                            st = qtr * 4 + st4
                            vstage = vs_pool.tile([P, H, HD + 1], f32r)
                            for nd in range(2):
                                ps = ps_pool.tile([P, SQ], f32)
                                for kt in range(DT):
                                    mm(
                                        ps[:],
                                        xt_q[:, kt, st4 * P : (st4 + 1) * P],
                                        wv[:, kt, nd * 512 : (nd + 1) * 512],
                                        start=(kt == 0),
                                        stop=(kt == DT - 1),
                                    )
                                vsl = vstage[:, nd * 8 : (nd + 1) * 8, 0:HD]
                                nc.vector.tensor_add(
                                    vsl, ps[:].rearrange("p (h c) -> p h c", c=HD),
                                    bv_sb[:, nd * 512 : (nd + 1) * 512].rearrange(
                                        "p (h c) -> p h c", c=HD
                                    ),
                                )
                                nc.vector.tensor_scalar_mul(
                                    vsl, in0=vsl, scalar1=em_sb[:, st : st + 1]
                                )
                            # denominator column: exp(mask) per ks row
                            nc.vector.tensor_copy(
                                vstage[:, :, HD : HD + 1],
                                em_sb[:, st : st + 1].to_broadcast((P, H, 1)),
                            )
                            nc.sync.dma_start(v_dram[st], vstage[:])

            wctx.__exit__(None, None, None)

            # ---------- Phase ATTN ----------
            with (
                tc.tile_pool(name="vh", bufs=3) as vh_pool,
                tc.tile_pool(name="expt", bufs=4) as ex_pool,
                tc.tile_pool(name="bcast", bufs=2) as bc_pool,
                tc.tile_pool(name="ps_sc", bufs=2, space="PSUM") as ps_sc,
                tc.tile_pool(name="ps_pv", bufs=2, space="PSUM") as ps_pv,
            ):
                for h in range(H):
                    t2, off = h // 2, (h % 2) * HD
                    v_h = vh_pool.tile([P, KS, HD + 1], f32r)
                    nc.sync.dma_start(
                        v_h[:], v_dram[:, :, h, :].rearrange("st p c -> p st c")
                    )
                    pv = ps_pv.tile([P, SQ], f32)
                    for jj in range(0, KS, 2):
                        sc = ps_sc.tile([P, 2 * SQ], f32)
                        for u in range(2):
                            j = jj + u
                            mm(
                                sc[:, u * SQ : (u + 1) * SQ],
                                kt_sb[off : off + HD, t2, j * P : (j + 1) * P],
                                qt_sb[off : off + HD, t2, :],
                                start=True,
                                stop=True,
                            )
                        ex = ex_pool.tile([P, 2 * SQ], f32r)
                        nc.scalar.activation(
                            ex[:], sc[:], mybir.ActivationFunctionType.Exp, scale=0.125
                        )
                        for u in range(2):
                            j = jj + u
                            mm(
                                pv[0 : HD + 1, :],
                                v_h[:, j, :],
                                ex[:, u * SQ : (u + 1) * SQ],
                                start=(j == 0),
                                stop=(j == KS - 1),
                            )
                    # normalize: ctxT = pv[0:64] * (1/pv[64]) broadcast over partitions
                    rec = bc_pool.tile([1, SQ], f32)
                    nc.vector.reciprocal(rec[:], pv[HD : HD + 1, :])
                    nc.sync.dma_start(sums_dram[h : h + 1, :], rec[:])
                    bcr = bc_pool.tile([HD, SQ], f32)
                    nc.sync.dma_start(
                        bcr[:], sums_dram[h : h + 1, :].to_broadcast((HD, SQ))
                    )
                    nc.vector.tensor_mul(
                        ctxn[off : off + HD, t2, :], pv[0:HD, :], bcr[:]
                    )

            # ---------- Phase PROJ + residual + LayerNorm ----------
            with (
                tc.tile_pool(name="wo_pool", bufs=1) as wo_pool,
                tc.tile_pool(name="lnconst", bufs=1) as lnc_pool,
                tc.tile_pool(name="xqp", bufs=1) as xq_pool,
                tc.tile_pool(name="xbuf", bufs=2) as xb_pool,
                tc.tile_pool(name="stats", bufs=4) as st_pool,
                tc.tile_pool(name="outp", bufs=3) as out_pool,
                tc.tile_pool(name="ps_o", bufs=4, space="PSUM") as ps_o,
            ):
                bo_sb = lnc_pool.tile([P, D], f32)
                nc.sync.dma_start(bo_sb[:], bo_bc[:])
                g_sb = lnc_pool.tile([P, D], f32)
                nc.sync.dma_start(g_sb[:], gamma_bc[:])
                be_sb = lnc_pool.tile([P, D], f32)
                nc.sync.dma_start(be_sb[:], beta_bc[:])
                eps_sb = lnc_pool.tile([P, 1], f32)
                nc.vector.memset(eps_sb[:], EPS)
                xq_sb = xq_pool.tile([P, 4, D], f32)
                nc.sync.dma_start(xq_sb[:], xq_r[:])
                wo = wo_pool.tile([P, DT, D], f32r)
                for dt in range(DT):
                    nc.sync.dma_start(wo[:, dt, :], W_r["o"][:, dt, :])

                for qp in range(4):
                    xbuf = xb_pool.tile([P, D], f32)
                    for nd in range(2):
                        ps = ps_o.tile([P, 512], f32)
                        for dt in range(DT):
                            mm(
                                ps[:],
                                ctxn[:, dt, qp * P : (qp + 1) * P],
                                wo[:, dt, nd * 512 : (nd + 1) * 512],
                                start=(dt == 0),
                                stop=(dt == DT - 1),
                            )
                        nsl = slice(nd * 512, (nd + 1) * 512)
                        nc.vector.tensor_add(xbuf[:, nsl], ps[:], bo_sb[:, nsl])
                        nc.vector.tensor_add(
                            xbuf[:, nsl], xbuf[:, nsl], xq_sb[:, qp, nsl]
                        )
                    # LayerNorm over the 1024 free elems
                    stats = st_pool.tile([P, 2, 6], f32)
                    xbuf_v = xbuf[:].rearrange("p (a d) -> p a d", a=2)
                    for a in range(2):
                        nc.vector.bn_stats(stats[:, a, :], xbuf_v[:, a, :])
                    mv = st_pool.tile([P, 2], f32)
                    nc.vector.bn_aggr(mv[:], stats[:])
                    rstd = st_pool.tile([P, 1], f32)
                    nc.scalar.activation(
                        rstd[:],
                        mv[:, 1:2],
                        mybir.ActivationFunctionType.Sqrt,
                        bias=eps_sb[:],
                    )
                    nc.vector.reciprocal(rstd[:], rstd[:])
                    nc.vector.tensor_scalar(
                        out=xbuf[:],
                        in0=xbuf[:],
                        scalar1=mv[:, 0:1],
                        scalar2=rstd[:],
                        op0=mybir.AluOpType.subtract,
                        op1=mybir.AluOpType.mult,
                    )
                    ot = out_pool.tile([P, D], f32)
                    nc.vector.tensor_mul(ot[:], xbuf[:], g_sb[:])
                    nc.vector.tensor_add(ot[:], ot[:], be_sb[:])
                    nc.sync.dma_start(out_r[qp], ot[:])

    nc.finalize()
    return nc


def _shard_inputs(inputs):
    """Build the 8 per-core input maps from full inputs."""
    x = np.ascontiguousarray(np.asarray(inputs["hidden_states"], dtype=np.float32))
    mask = np.asarray(inputs["attention_mask"], dtype=np.float32).reshape(B, S)
    W = {k: np.ascontiguousarray(np.asarray(inputs[k], dtype=np.float32))
         for k in ("Wq", "Wk", "Wv", "Wo")}
    bq = np.asarray(inputs["bq"], dtype=np.float32)
    bk = np.asarray(inputs["bk"], dtype=np.float32)
    bv = np.asarray(inputs["bv"], dtype=np.float32)
    bo = np.asarray(inputs["bo"], dtype=np.float32)
    gamma = np.asarray(inputs["ln_gamma"], dtype=np.float32)
    beta = np.asarray(inputs["ln_beta"], dtype=np.float32)

    bq_t = np.ascontiguousarray(bq.reshape(DT, P).T)
    bk_t = np.ascontiguousarray(bk.reshape(DT, P).T)
    bv_bc = np.ascontiguousarray(np.broadcast_to(bv, (P, D)))
    bo_bc = np.ascontiguousarray(np.broadcast_to(bo, (P, D)))
    gamma_bc = np.ascontiguousarray(np.broadcast_to(gamma, (P, D)))
    beta_bc = np.ascontiguousarray(np.broadcast_to(beta, (P, D)))

    xTb = [np.ascontiguousarray(x[b].T) for b in range(B)]
    em_t = [np.ascontiguousarray(np.exp(mask[b]).reshape(KS, P).T) for b in range(B)]

    in_maps = []
    for c in range(NCORES):
        b, q = c // 4, (c % 4) * SQ
        in_maps.append(
            {
                "xT": xTb[b],
                "xqT": np.ascontiguousarray(xTb[b][:, q : q + SQ]),
                "xq": np.ascontiguousarray(x[b, q : q + SQ, :]),
                "Wq": W["Wq"], "Wk": W["Wk"], "Wv": W["Wv"], "Wo": W["Wo"],
                "bq_t": bq_t, "bk_t": bk_t,
                "bv_bc": bv_bc, "bo_bc": bo_bc,
                "gamma_bc": gamma_bc, "beta_bc": beta_bc,
                "emask_t": em_t[b],
            }
        )
    return in_maps


def run(inputs, trace=False, **kw):
    """Run on hardware; returns (full_output, BassKernelResults)."""
    _ensure_paths()
    from concourse.bass_utils import run_bass_kernel_spmd

    if "nc" not in _CACHE:
        _CACHE["nc"] = build_nc()
    nc = _CACHE["nc"]
    in_maps = _shard_inputs(inputs)
    res = run_bass_kernel_spmd(
        nc, in_maps, core_ids=list(range(NCORES)), trace=trace, **kw
    )
    parts = [res.results[c]["out"] for c in range(NCORES)]
    full = np.empty((B, S, D), dtype=np.float32)
    for c in range(NCORES):
        b, q = c // 4, (c % 4) * SQ
        full[b, q : q + SQ] = parts[c]
    return full, res


def kernel(**inputs):
    out, _ = run(inputs)
    return out
